# revision 48
# baseline (speedup 1.0000x reference)
"""Trainium2 Bass kernel for nn_CLRerHead (CLRNet-style lane-detection head).

Sharding: data-parallel over batch. 32 batch items -> 8 cores x 4 items each.
Each core runs the full 3-level refinement for its 4 items.

Gather strategy ("negative tent" matmul gather): per level,
  grid_sample + roi-flatten + FC fuse into two PE matmul stages:
    RCPROJ_s (W, 64) = stackfeat_s(128, W).T @ nwfc2_s(128, 64)
      (y0/y1 corner rows stacked in K, -wy weights folded into nwfc2)
    f (64, 192)     += sum_G matmul(lhsT=RCPROJ_G(SR*W, 64), rhs=v_G)
  with v = min(|x - w| - 1, 0) the negative tent (signs cancel), built by
    PE broadcast matmul (block-diag ones lhsT over [hi;lo] fp16 rows of
    clamped x) -> ACT Abs(psum + (-iota mod W) bias) -> DVE min -> fp16.
  SR in {4,2,1} s-values stack into the 128 partitions per level, cutting
  tent elem-work and matmul rows by SR.
All feature/attention/head matmuls run fp16 operands with fp32 PSUM.
"""

import math
import numpy as np
import ml_dtypes
from contextlib import ExitStack

import concourse.bass as bass
import concourse.bacc as bacc
import concourse.mybir as mybir
import concourse.tile as tile
from concourse import bass_utils

dt = mybir.dt
AF = mybir.ActivationFunctionType
ALU = mybir.AluOpType

# ---------------- static problem config ----------------
IMG_W, IMG_H = 800.0, 320.0
NR, NS, NP, FC = 72, 36, 192, 64
N_STRIPS = NR - 1
ALPHA = IMG_H / IMG_W
SAMPLE_IDX = (np.linspace(0.0, 1.0, NS) * N_STRIPS).astype(np.int64)
PRIOR_FEAT_YS = np.flip(SAMPLE_IDX.astype(np.float32) / N_STRIPS).copy()
PRIOR_YS = np.linspace(1.0, 0.0, NR, dtype=np.float32)

N_CORES = 8
NB = 4
# (H, W, SR, WP): feat2, feat1, feat0; SR s-values stacked at stride WP
LEVELS = [(10, 25, 4, 32), (20, 50, 2, 64), (40, 100, 1, 100)]
PCH = [(0, 128), (128, 64)]
FP16 = dt.float16
F32 = dt.float32
NPTS = NP * NS          # 6912 tent points per (item, level)
CH = 512                # tent chunk free-width (one psum bank)
SG = 8                  # s-values per rc psum tile

Q_S = (1.0 - PRIOR_YS[SAMPLE_IDX[::-1]]).astype(np.float32)
QF_R = (1.0 - PRIOR_YS).astype(np.float32)


def _perm(SR):
    return [s for j in range(SR) for s in range(j, NS, SR)]


def _level_ytab(H):
    ys = PRIOR_FEAT_YS * (H - 1)
    y0 = np.clip(np.floor(ys).astype(np.int64), 0, H - 1)
    y1 = np.minimum(y0 + 1, H - 1)
    wy1 = (ys - y0).astype(np.float32)
    wy1 = np.where(y1 == y0, 0.0, wy1).astype(np.float32)
    wy0 = (1.0 - wy1).astype(np.float32)
    return y0, y1, wy0, wy1


def _neg_wywfc2(W_fc, H):
    """(128, NS*64): rows 0:64 = -wy0(s)*Wfc_s, rows 64:128 = -wy1(s)*Wfc_s."""
    _, _, wy0, wy1 = _level_ytab(H)
    out = np.zeros((128, NS * 64), np.float32)
    for s in range(NS):
        out[0:64, s * 64:(s + 1) * 64] = -wy0[s] * W_fc[s::NS, :]
        out[64:128, s * 64:(s + 1) * 64] = -wy1[s] * W_fc[s::NS, :]
    return out.astype(np.float16)


def _stackfeat_all(f, H, W, WP):
    """(128, NB*NS*WP): per item blocks of [feat[:, y0(s), :]; feat[:, y1(s)]],
    zero-padded to WP columns so PE pad rows compute to zero."""
    y0, y1, _, _ = _level_ytab(H)
    f16 = f.astype(np.float16)                      # (NB, 64, H, W)
    out = np.zeros((128, NB, NS, WP), np.float16)
    out[0:64, :, :, 0:W] = f16[:, :, y0, :].transpose(1, 0, 2, 3)
    out[64:128, :, :, 0:W] = f16[:, :, y1, :].transpose(1, 0, 2, 3)
    return out.reshape(128, NB * NS * WP)


# ---------------- packed const layouts (static) ----------------
def _pack_layout(entries):
    off = 0
    lay = {}
    for name, rows, cols in entries:
        lay[name] = (rows, off, cols)
        off += cols
    return lay, off


PACK32 = [
    ("sinargsT", 64, NB), ("W_t1", 64, 256), ("b_t1", 128, 2),
    ("W_t2a", 128, 256), ("W_t2b", 128, 256), ("b_t2", 128, 2),
    ("W_sta", 128, 128), ("W_stb", 128, 128), ("bstS1", 64, 1),
    ("bstSh", 64, 1), ("W_tca", 128, 64), ("W_tcb", 128, 64), ("b_tc", 64, 1),
    ("b_fc", 64, 1), ("b_c1", 64, 1), ("b_c2", 64, 1), ("b_r1", 64, 1),
    ("b_r2", 64, 1), ("b_cls", 2, 1), ("b_reg", 76, 1),
    ("W_r1f", 64, 64), ("W_r2f", 64, 64), ("W_regf", 64, 76),
    ("qrep0", 128, NS), ("qrep1", 128, NS), ("qrep2", 128, NS),
    ("qfrep", 128, NR), ("negiota0", 128, 1), ("negiota1", 128, 1),
    ("negiota2", 128, 1), ("halfpi", 128, 1), ("ident", 128, 128),
]
LAY32, COLS32 = _pack_layout(PACK32)

PACK16 = [
    ("W_q", 64, 64), ("W_k", 64, 64), ("W_v", 64, 64), ("W_o_bf", 64, 64),
    ("W_c1", 64, 64), ("W_c2", 64, 64), ("W_r1", 64, 64), ("W_r2", 64, 64),
    ("W_cls", 64, 2), ("W_reg", 64, 76), ("ones_bf", 128, 1),
    ("onesbc0", 8, 128), ("onesbc1", 4, 128), ("onesbc2", 2, 128),
]
LAY16, COLS16 = _pack_layout(PACK16)

_CACHE = {}


def _build_program(num_devices=N_CORES):
    nc = bacc.Bacc("TRN2", target_bir_lowering=False, debug=False,
                   num_devices=num_devices)
    D = {}

    def din(name, shape, dtype=F32):
        D[name] = nc.dram_tensor(name, list(shape), dtype, kind="ExternalInput")

    for li, (H, W, SR, WP) in enumerate(LEVELS):
        din(f"sfeat{li}", (128, NB * NS * WP), FP16)
        din(f"nwfc{li}", (128, NS * 64), FP16)
    din("cpack32", (128, COLS32))
    din("cpack16", (128, COLS16), FP16)
    din("anchA", (128, NB * 3))
    din("anchB", (64, NB * 3))

    out_t = nc.dram_tensor("out", [NB, NP, 78], F32, kind="ExternalOutput")

    with tile.TileContext(nc) as tc, ExitStack() as ex:
        cpool = ex.enter_context(tc.tile_pool(name="consts", bufs=1))
        state = ex.enter_context(tc.tile_pool(name="state", bufs=1))
        wk = ex.enter_context(tc.tile_pool(name="work", bufs=2))
        big = ex.enter_context(tc.tile_pool(name="big", bufs=2))
        ps = ex.enter_context(tc.tile_pool(name="ps", bufs=2, space="PSUM"))
        psf = ex.enter_context(tc.tile_pool(name="psf", bufs=2, space="PSUM"))
        psrc = ex.enter_context(tc.tile_pool(name="psrc", bufs=2, space="PSUM"))
        psbc = ex.enter_context(tc.tile_pool(name="psbc", bufs=2, space="PSUM"))

        C = {}
        SF, NW = {}, {}
        for li, (H, W, SR, WP) in enumerate(LEVELS):
            SF[li] = cpool.tile([128, NB * NS * WP], FP16, tag=f"sfeat{li}",
                                name=f"c_sfeat{li}")
            NW[li] = cpool.tile([128, NS * 64], FP16, tag=f"nwfc{li}",
                                name=f"c_nwfc{li}")
        cp32 = cpool.tile([128, COLS32], F32, tag="cpack32", name="cp32")
        cp16 = cpool.tile([128, COLS16], FP16, tag="cpack16", name="cp16")
        # DMA order: level-0-critical tensors first so compute starts early.
        nc.sync.dma_start(cp32[:], D["cpack32"].ap())
        nc.sync.dma_start(cp16[:], D["cpack16"].ap())
        nc.scalar.dma_start(NW[0][:], D["nwfc0"].ap())
        nc.scalar.dma_start(SF[0][:], D["sfeat0"].ap())
        for li in (1, 2):
            nc.scalar.dma_start(SF[li][:], D[f"sfeat{li}"].ap())
            nc.scalar.dma_start(NW[li][:], D[f"nwfc{li}"].ap())
        for name, (rows, off, cols) in LAY32.items():
            C[name] = cp32[0:rows, off:off + cols]
        for name, (rows, off, cols) in LAY16.items():
            C[name] = cp16[0:rows, off:off + cols]

        anchT = {}
        anchT[0] = state.tile([128, NB * 3], F32, tag="anchA", name="anchA_t")
        anchT[1] = state.tile([64, NB * 3], F32, tag="anchB", name="anchB_t")
        nc.sync.dma_start(anchT[0][:], D["anchA"].ap())
        nc.sync.dma_start(anchT[1][:], D["anchB"].ap())
        anch = {(b, ci): anchT[ci][:, b * 3:(b + 1) * 3]
                for b in range(NB) for ci in range(2)}



        # ---------------- time MLP (emitted in 4 parts, interleaved
        # with level-0 passT so its serial chain hides behind tent work) ----
        MS = {}

        def mlp_part1():
            sinT = wk.tile([64, NB], F32, tag="tm_sin", name="sinT")
            nc.scalar.activation(sinT[:], C["sinargsT"][:], AF.Sin)
            emb = []
            for m in range(2):
                p = ps.tile([128, NB], F32, tag="mm", name=f"p_emb{m}")
                nc.tensor.matmul(p[:], C["W_t1"][:, m * 128:(m + 1) * 128],
                                 sinT[:])
                x = state.tile([128, NB], F32, tag=f"emb{m}", name=f"emb{m}")
                nc.scalar.activation(x[:], p[:], AF.Identity,
                                     bias=C["b_t1"][:, m:m + 1])
                emb.append(x)
            MS["emb"] = emb

        def mlp_part2():
            emb = MS["emb"]
            for m in range(2):
                x = emb[m]
                sq = wk.tile([128, NB], F32, tag="tm_sq", name=f"sq{m}")
                nc.scalar.activation(sq[:], x[:], AF.Square)
                cu = wk.tile([128, NB], F32, tag="tm_cu", name=f"cu{m}")
                nc.vector.tensor_tensor(cu[:], sq[:], x[:], ALU.mult)
                nc.vector.tensor_scalar(cu[:], cu[:], 0.044715, None, ALU.mult)
                nc.vector.tensor_tensor(cu[:], cu[:], x[:], ALU.add)
                th = wk.tile([128, NB], F32, tag="tm_th", name=f"th{m}")
                nc.scalar.activation(th[:], cu[:], AF.Tanh,
                                     scale=float(np.sqrt(2.0 / np.pi)))
                nc.vector.tensor_scalar(th[:], th[:], 1.0, 0.5, ALU.add,
                                        ALU.mult)
                nc.vector.tensor_tensor(x[:], th[:], x[:], ALU.mult)

        def mlp_part3():
            emb = MS["emb"]
            tmb = []
            for m in range(2):
                p = ps.tile([128, NB], F32, tag="mm", name=f"p_tmb{m}")
                for k in range(2):
                    wt2 = C["W_t2a"] if k == 0 else C["W_t2b"]
                    nc.tensor.matmul(p[:], wt2[:, m * 128:(m + 1) * 128],
                                     emb[k][:], start=(k == 0), stop=(k == 1))
                x = state.tile([128, NB], F32, tag=f"tmb{m}", name=f"tmb{m}")
                nc.scalar.activation(x[:], p[:], AF.Identity,
                                     bias=C["b_t2"][:, m:m + 1])
                tmb.append(x)
            sil = []
            for m in range(2):
                # silu(x) = 0.5*x*(1 + tanh(x/2)) keeps the ACT stream inside
                # the silu/tanh/sin table set (no sigmoid-set load).
                sl = wk.tile([128, NB], F32, tag=f"tm_sil{m}", name=f"sil{m}")
                nc.scalar.activation(sl[:], tmb[m][:], AF.Tanh, scale=0.5)
                nc.vector.tensor_scalar(sl[:], sl[:], 1.0, 0.5, ALU.add,
                                        ALU.mult)
                nc.vector.tensor_tensor(sl[:], sl[:], tmb[m][:], ALU.mult)
                sil.append(sl)
            MS["tmb"], MS["sil"] = tmb, sil

        def mlp_part4():
            tmb, sil = MS["tmb"], MS["sil"]
            for j, (dst, bias) in enumerate([(scale1T, "bstS1"),
                                             (shiftT, "bstSh")]):
                p = ps.tile([64, NB], F32, tag="mm", name=f"p_ss{j}")
                for k in range(2):
                    wst = C["W_sta"] if k == 0 else C["W_stb"]
                    nc.tensor.matmul(p[:], wst[:, j * 64:(j + 1) * 64],
                                     sil[k][:], start=(k == 0), stop=(k == 1))
                nc.scalar.activation(dst[:], p[:], AF.Identity,
                                     bias=C[bias][:, 0:1])
            ptk = ps.tile([64, NB], F32, tag="mm", name="p_tok")
            for k in range(2):
                wtc = C["W_tca"] if k == 0 else C["W_tcb"]
                nc.tensor.matmul(ptk[:], wtc[:], tmb[k][:], start=(k == 0),
                                 stop=(k == 1))
            nc.scalar.activation(tokT[:], ptk[:], AF.Identity,
                                 bias=C["b_tc"][:, 0:1])

        scale1T = state.tile([64, NB], F32, tag="scale1T", name="scale1T")
        shiftT = state.tile([64, NB], F32, tag="shiftT", name="shiftT")
        tokT = state.tile([64, NB], F32, tag="tokT", name="tokT")

        # ---------------- helpers ----------------
        def gen_ab(b, scaleW, tagsfx, gate=None):
            res = []
            for ci, (p0, pn) in enumerate(PCH):
                A = anch[(b, ci)]
                th = A[:, 2:3]
                if gate is not None:
                    tg = wk.tile([pn, 1], F32, tag=f"tg{ci}{tagsfx}",
                                 name=f"tg{b}{ci}")
                    nc.vector.tensor_tensor(tg[:], A[:, 2:3], gate[0:pn, 0:1],
                                            ALU.add)
                    th = tg[:, 0:1]
                sn = wk.tile([pn, 1], F32, tag=f"sn{ci}{tagsfx}", name=f"sn{b}{ci}")
                cs = wk.tile([pn, 1], F32, tag=f"cs{ci}{tagsfx}", name=f"cs{b}{ci}")
                nc.scalar.activation(sn[:], th, AF.Sin, scale=math.pi)
                nc.scalar.activation(cs[:], th, AF.Sin, scale=-math.pi,
                                     bias=C["halfpi"][0:pn, 0:1])
                g = wk.tile([pn, 1], F32, tag=f"g{ci}{tagsfx}", name=f"g{b}{ci}")
                nc.vector.reciprocal(g[:], sn[:])
                nc.vector.tensor_tensor(g[:], cs[:], g[:], ALU.mult)
                nc.vector.tensor_scalar(g[:], g[:], 1000.0, -1000.0,
                                        ALU.min, ALU.max)
                nc.vector.tensor_scalar(g[:], g[:], ALPHA, None, ALU.mult)
                base = wk.tile([pn, 1], F32, tag=f"bs{ci}{tagsfx}", name=f"bs{b}{ci}")
                nc.vector.tensor_tensor(base[:], A[:, 0:1], g[:], ALU.mult)
                nc.vector.tensor_tensor(base[:], A[:, 1:2], base[:], ALU.subtract)
                aC = wk.tile([pn, 1], F32, tag=f"aC{ci}{tagsfx}", name=f"aC{b}{ci}")
                bC = wk.tile([pn, 1], F32, tag=f"bC{ci}{tagsfx}", name=f"bC{b}{ci}")
                nc.vector.tensor_scalar(aC[:], base[:], scaleW, None, ALU.mult)
                nc.vector.tensor_scalar(bC[:], g[:], scaleW, None, ALU.mult)
                res.append((aC, bC, base, g))
            return res

        # ---------------- main loop ----------------
        def emit_prefix(li, b, gate=None):
            """gen_ab + xf + transpose + clamp/split + rhsS collapse DMAs."""
            H, W, SR, WP = LEVELS[li]
            qrep = C[f"qrep{li}"]
            xfs = {}
            ab = gen_ab(b, float(W - 1), "a", gate=gate)
            for ci, (p0, pn) in enumerate(PCH):
                aC, bC, _, _ = ab[ci]
                xf = state.tile([pn, NS], F32, tag=f"xf{b}_{ci}",
                                name=f"xf{b}_{ci}_{li}")
                nc.vector.tensor_scalar(xf[:], qrep[0:pn, :],
                                        bC[:, 0:1], aC[:, 0:1],
                                        ALU.mult, ALU.add)
                xfs[ci] = xf
            ptx = ps.tile([NS, NP], F32, tag="mm", name=f"ptx{b}")
            for ci, (p0, pn) in enumerate(PCH):
                nc.tensor.transpose(ptx[:, p0:p0 + pn], xfs[ci][:],
                                    C["ident"][0:pn, 0:pn])
            xfc = wk.tile([NS, NP], F32, tag="xfc", bufs=3, name=f"xfc{b}")
            nc.vector.tensor_scalar(xfc[:], ptx[:], -2.0, float(W + 2),
                                    ALU.max, ALU.min)
            hi16 = wk.tile([NS, NP], FP16, tag="hi16", bufs=3, name=f"hi16{b}")
            nc.gpsimd.tensor_copy(hi16[:], xfc[:])
            lo16 = wk.tile([NS, NP], FP16, tag="lo16", bufs=3, name=f"lo16{b}")
            nc.gpsimd.tensor_tensor(lo16[:], xfc[:], hi16[:], ALU.subtract)
            rhsS = big.tile([2 * SR, NPTS // SR], FP16, tag="rhsS", bufs=4,
                            name=f"rhsS{b}")
            nc.sync.dma_start(rhsS[0:SR, :], hi16[:])
            nc.sync.dma_start(rhsS[SR:2 * SR, :], lo16[:])
            return rhsS

        pre = {(0, b): emit_prefix(0, b) for b in range(NB)}

        for li, (H, W, SR, WP) in enumerate(LEVELS):
            is_last = li == len(LEVELS) - 1
            sfeat, nwfc = SF[li], NW[li]
            NQL = NPTS // SR      # tent columns (1728/3456/6912)
            NGRP = NS // SR       # fps groups (9/18/36)
            nch = (NQL + CH - 1) // CH
            nio = C[f"negiota{li}"]
            obc = C[f"onesbc{li}"]

            # pass T: tent + PROJ + fps + fT per item
            fTs = {}
            gates = {}
            def passT_item(b):
                rhsS = pre.pop((li, b))
                # PROJ: one corner-stacked matmul per s into rc psum tiles
                rcs = []
                nloc = SG // SR   # fps groups per rc tile (2/4/8)
                for k in range((NS + SG - 1) // SG):
                    smax = min((k + 1) * SG, NS)
                    qlmax = (smax - 1) // SR - k * nloc + 1
                    rc = psrc.tile([128, SG * 64], F32, tag="rc_ps",
                                   name=f"rc{b}_{k}")
                    for s in range(k * SG, smax):
                        j = s % SR
                        ql = s // SR - k * nloc
                        nc.tensor.matmul(
                            rc[j * WP:j * WP + WP, ql * 64:(ql + 1) * 64],
                            sfeat[:, (b * NS + s) * WP:(b * NS + s + 1) * WP],
                            nwfc[:, s * 64:(s + 1) * 64],
                            tile_position=(0, j * WP))
                    rcsb = wk.tile([128, SG * 64 // SR], FP16, tag="rcsb",
                                   bufs=6, name=f"rcsb{b}_{k}")
                    nc.vector.tensor_copy(rcsb[0:SR * WP, 0:qlmax * 64],
                                          rc[0:SR * WP, 0:qlmax * 64])
                    rcs.append(rcsb)

                # tent chunks: PE bcast -> ACT abs -> DVE min, 512-wide
                # (full psum bank); min lands in one contiguous v16 tile so
                # fps can slice at any 192 offset.
                v16 = big.tile([128, NPTS], FP16, tag="v16", bufs=2,
                               name=f"v16{b}")
                for c in range(nch):
                    cw = min(CH, NQL - c * CH)
                    bc = psbc.tile([128, CH], F32, tag="bc", name=f"bc{b}_{c}")
                    nc.tensor.matmul(bc[0:SR * WP, 0:cw],
                                     obc[0:2 * SR, 0:SR * WP],
                                     rhsS[0:2 * SR, c * CH:c * CH + cw])
                    d16 = wk.tile([128, CH], FP16, tag="d16", bufs=3,
                                  name=f"d16{b}_{c}")
                    nc.scalar.activation(d16[0:SR * WP, 0:cw], bc[0:SR * WP, 0:cw],
                                         AF.Abs, bias=nio[0:SR * WP, 0:1])
                    nc.vector.tensor_scalar(v16[0:SR * WP, c * CH:c * CH + cw],
                                            d16[0:SR * WP, 0:cw], 1.0, 0.0,
                                            ALU.subtract, ALU.min)

                # fps accumulation over groups
                fps = psf.tile([64, NP], F32, tag="f_ps", name=f"fps{b}")
                for G in range(NGRP):
                    k = G // nloc
                    ql = G % nloc
                    nc.tensor.matmul(
                        fps[:],
                        rcs[k][0:SR * WP, ql * 64:(ql + 1) * 64],
                        v16[0:SR * WP, G * NP:(G + 1) * NP],
                        start=(G == 0), stop=(G == NGRP - 1))

                fT = wk.tile([64, NP], F32, tag="fT", bufs=5, name=f"fT{b}")
                nc.scalar.activation(fT[:], fps[:], AF.Relu, bias=C["b_fc"][:, 0:1])
                fTs[b] = fT

            # pass A: attention per item (psum packed into 4 tiles)
            def passA_item(b):
                fT = fTs[b]
                nc.vector.tensor_scalar(fT[:], fT[:], tokT[:, b:b + 1], None,
                                        ALU.add)
                fT16 = wk.tile([64, NP], FP16, tag="fT16", bufs=5,
                               name=f"fT16{b}")
                nc.gpsimd.tensor_copy(fT16[:], fT[:])
                fTs[b] = (fT, fT16)
                qk = ps.tile([128, NP], F32, tag="mm", name=f"qk{b}")
                nc.tensor.matmul(qk[0:64, :], C["W_q"][:], fT16[:])
                nc.tensor.matmul(qk[64:128, :], C["W_k"][:], fT16[:],
                                 tile_position=(0, 64))
                qT = wk.tile([64, NP], FP16, tag="qT", name=f"qT{b}")
                nc.vector.tensor_scalar(qT[:], qk[0:64, :], 0.125, None,
                                        ALU.mult)
                kT = wk.tile([64, NP], FP16, tag="kT", name=f"kT{b}")
                nc.vector.tensor_copy(kT[:], qk[64:128, :])
                vp = ps.tile([128, 128], F32, tag="mm", name=f"vp{b}")
                nc.tensor.matmul(vp[0:128, 0:64], fT16[:, 0:128], C["W_v"][:])
                nc.tensor.matmul(vp[0:64, 64:128], fT16[:, 128:192], C["W_v"][:])
                vn = []
                for ci, (p0, pn) in enumerate(PCH):
                    vt = wk.tile([pn, 64], FP16, tag=f"vn{ci}", name=f"vn{b}{ci}")
                    nc.vector.tensor_copy(vt[:], vp[0:pn, ci * 64:ci * 64 + 64])
                    vn.append(vt)
                sp = ps.tile([128, 2 * NP], F32, tag="mm", name=f"sp{b}")
                nc.tensor.matmul(sp[0:128, 0:NP], kT[:, 0:128], qT[:])
                nc.tensor.matmul(sp[0:64, NP:2 * NP], kT[:, 128:192], qT[:])
                est = []
                for ci, (p0, pn) in enumerate(PCH):
                    e = wk.tile([pn, NP], FP16, tag=f"est{ci}", name=f"est{b}{ci}")
                    nc.scalar.activation(e[:], sp[0:pn, ci * NP:ci * NP + NP],
                                         AF.Exp)
                    est.append(e)
                zav = ps.tile([128, 2 * NP], F32, tag="mm", name=f"zav{b}")
                for ci, (p0, pn) in enumerate(PCH):
                    nc.tensor.matmul(zav[0:1, 0:NP], C["ones_bf"][0:pn, 0:1],
                                     est[ci][:], start=(ci == 0), stop=(ci == 1))
                rrow = wk.tile([1, NP], F32, tag="rrow", name=f"rrow{b}")
                nc.vector.reciprocal(rrow[:], zav[0:1, 0:NP])
                rbc = wk.tile([64, NP], F32, tag="rbc", name=f"rbc{b}")
                nc.gpsimd.partition_broadcast(rbc[:], rrow[0:1, :], channels=64)
                for ci in range(2):
                    nc.tensor.matmul(zav[64:128, 0:NP], vn[ci][:], est[ci][:],
                                     start=(ci == 0), stop=(ci == 1),
                                     tile_position=(0, 64))
                avsb = wk.tile([64, NP], FP16, tag="avsb", name=f"avsb{b}")
                nc.vector.tensor_copy(avsb[:], zav[64:128, 0:NP])
                nc.tensor.matmul(zav[0:64, NP:2 * NP], C["W_o_bf"][:], avsb[:])
                t1 = wk.tile([64, NP], F32, tag="attnt", name=f"t1{b}")
                nc.vector.tensor_tensor(t1[:], zav[0:64, NP:2 * NP], rbc[:],
                                        ALU.mult)
                nc.vector.tensor_tensor(fT[:], fT[:], t1[:], ALU.add)
                # FiLM
                nc.vector.tensor_scalar(fT[:], fT[:], scale1T[:, b:b + 1],
                                        shiftT[:, b:b + 1], ALU.mult, ALU.add)
                gates[b] = est[0]

            # pass H: heads per item (+ next level prefix / final outputs)
            def passH_item(b):
                fT, _ = fTs[b]

                def head_mm(wname, bias, src, relu=True, out_p=64, out_dt=FP16):
                    p = ps.tile([128, NP], F32, tag="mm", name=f"p_{wname}{b}")
                    nc.tensor.matmul(p[0:out_p, :], C[wname][:], src[:])
                    o = wk.tile([out_p, NP], out_dt, tag=f"hd_{wname}",
                                name=f"{wname}o{b}")
                    nc.scalar.activation(o[:], p[0:out_p, :],
                                         AF.Relu if relu else AF.Identity,
                                         bias=C[bias][:, 0:1])
                    return o

                pr = ps.tile([128, NP], F32, tag="mm", name=f"pr{b}")
                nc.tensor.matmul(pr[0:64, :], C["W_r1f"][:], fT[:])
                r1 = wk.tile([64, NP], F32, tag="hd_r1", name=f"r1o{b}")
                nc.vector.tensor_scalar(r1[:], pr[0:64, :], C["b_r1"][:, 0:1],
                                        0.0, ALU.add, ALU.max)
                nc.tensor.matmul(pr[64:128, :], C["W_r2f"][:], r1[:],
                                 tile_position=(0, 64))
                r2 = wk.tile([64, NP], F32, tag="hd_r2", name=f"r2o{b}")
                nc.vector.tensor_scalar(r2[:], pr[64:128, :], C["b_r2"][:, 0:1],
                                        0.0, ALU.add, ALU.max)
                # non-last levels only consume reg[..., 0:3] (anchor
                # deltas): shrink the reg head to 3 columns and update
                # anchors straight from the transpose psum (no rn copy).
                nreg = 76 if is_last else 3
                p_regT = ps.tile([128, NP], F32, tag="mm", name=f"p_regT{b}")
                nc.tensor.matmul(p_regT[0:nreg, :], C["W_regf"][:, 0:nreg],
                                 r2[:])
                regT = wk.tile([nreg, NP], F32, tag="hd_regf",
                               name=f"regTo{b}")
                nc.scalar.activation(regT[:], p_regT[0:nreg, :], AF.Identity,
                                     bias=C["b_reg"][0:nreg, 0:1])

                rns = {}
                pt = ps.tile([128, 2 * 76], F32, tag="mm", name=f"p_rt{b}")
                for ci, (p0, pn) in enumerate(PCH):
                    nc.tensor.transpose(pt[0:pn, ci * nreg:ci * nreg + nreg],
                                        regT[:, p0:p0 + pn],
                                        C["ident"][0:nreg, 0:nreg])
                    A = anch[(b, ci)]
                    if is_last:
                        rn = state.tile([pn, 76], F32, tag=f"regn{b}_{ci}",
                                        name=f"regn{b}_{ci}_{li}")
                        nc.vector.tensor_copy(rn[:],
                                              pt[0:pn, ci * 76:ci * 76 + 76])
                        nc.vector.tensor_tensor(A[:, :], A[:, :], rn[:, 0:3],
                                                ALU.add)
                        rns[ci] = rn
                    else:
                        nc.vector.tensor_tensor(
                            A[:, :], A[:, :],
                            pt[0:pn, ci * nreg:ci * nreg + 3], ALU.add)

                if is_last:
                    stg = {ci: state.tile([pn, 78], F32, tag=f"stg{b}{ci}",
                                          name=f"stg{b}{ci}")
                           for ci, (p0, pn) in enumerate(PCH)}
                    fT16f = wk.tile([64, NP], FP16, tag="fT16f",
                                    name=f"fT16f{b}")
                    nc.gpsimd.tensor_copy(fT16f[:], fT[:])
                    c1 = head_mm("W_c1", "b_c1", fT16f)
                    c2 = head_mm("W_c2", "b_c2", c1)
                    clsT = head_mm("W_cls", "b_cls", c2, relu=False, out_p=2,
                                   out_dt=F32)
                    ab = gen_ab(b, 1.0, "o")
                    ptc = ps.tile([128, 4], F32, tag="mm", name=f"p_ct{b}")
                    for ci, (p0, pn) in enumerate(PCH):
                        nc.tensor.transpose(ptc[0:pn, ci * 2:ci * 2 + 2],
                                            clsT[:, p0:p0 + pn],
                                            C["ident"][0:2, 0:2])
                        nc.vector.tensor_copy(stg[ci][:, 0:2],
                                              ptc[0:pn, ci * 2:ci * 2 + 2])
                        A = anch[(b, ci)]
                        rn = rns[ci]
                        nc.gpsimd.tensor_copy(stg[ci][:, 2:5], A[:, :])
                        nc.gpsimd.tensor_copy(stg[ci][:, 5:6], rn[:, 3:4])
                        _, _, base, g = ab[ci]
                        nc.vector.tensor_scalar(stg[ci][:, 6:78],
                                                C["qfrep"][0:pn, 0:NR],
                                                g[:, 0:1], base[:, 0:1],
                                                ALU.mult, ALU.add)
                        nc.vector.tensor_tensor(stg[ci][:, 6:78],
                                                stg[ci][:, 6:78], rn[:, 4:76],
                                                ALU.add)
                        nc.sync.dma_start(out_t.ap()[b, p0:p0 + pn, 0:78],
                                          stg[ci][:])

            # interleaved schedule: T0 T1 A0 T2 A1 T3 A2 H0 A3 H1 H2 H3
            if li == 0:
                mlp_part1()
                passT_item(0); mlp_part2()
                passT_item(1); mlp_part3()
                passT_item(2); mlp_part4()
                passT_item(3)
                passA_item(0)
                passA_item(1)
                passA_item(2)
            else:
                passT_item(0); passT_item(1)
                passA_item(0)
                passT_item(2)
                passA_item(1)
                passT_item(3)
                passA_item(2)
            passH_item(0)
            passA_item(3)
            passH_item(1)
            if not is_last:
                ez = wk.tile([128, 1], F32, tag="ezgate", name=f"ez{li}")
                nc.vector.tensor_scalar(ez[:], gates[3][0:128, 0:1], 0.0, None,
                                        ALU.mult)
                pre[(li + 1, 0)] = emit_prefix(li + 1, 0, gate=ez)
                pre[(li + 1, 1)] = emit_prefix(li + 1, 1, gate=ez)
            passH_item(2); passH_item(3)
            if not is_last:
                pre[(li + 1, 2)] = emit_prefix(li + 1, 2, gate=ez)
                pre[(li + 1, 3)] = emit_prefix(li + 1, 3, gate=ez)

    nc.compile()
    _CACHE.pop("regn", None)
    return nc


def _host_inputs(inp_slice, nwfc2, shared):
    m = dict(shared)
    for li, key in enumerate(["feat2", "feat1", "feat0"]):
        f = np.asarray(inp_slice[key], np.float32)
        H, W, SR, WP = LEVELS[li]
        m[f"sfeat{li}"] = _stackfeat_all(f, H, W, WP)
        m[f"nwfc{li}"] = nwfc2[li]
    w = {k: np.asarray(v, np.float32) for k, v in inp_slice.items()
         if k.startswith(("W_", "b_"))}

    half = FC // 2
    freqs = np.exp(np.arange(half, dtype=np.float32)
                   * (-math.log(10000.0) / (half - 1)))
    ang = np.asarray(inp_slice["t"]).astype(np.float32)[:, None] * freqs[None, :]
    full = np.concatenate([ang, ang + math.pi / 2.0], axis=1)
    full = np.mod(full + math.pi, 2.0 * math.pi) - math.pi

    if "cp32base" in shared:
        m.pop("cp32base", None)
        cp32 = shared["cp32base"].copy()
        rows, off, cols = LAY32["sinargsT"]
        cp32[0:rows, off:off + cols] = np.ascontiguousarray(full.T)
        m["cpack32"] = cp32
        m["anchA"], m["anchB"] = _anch_pack(inp_slice)
        return {k: np.ascontiguousarray(np.asarray(v)) for k, v in m.items()}

    v32 = {}
    v32["sinargsT"] = np.ascontiguousarray(full.T)
    v32["W_t1"] = w["W_t1"]
    v32["b_t1"] = np.ascontiguousarray(w["b_t1"].reshape(2, 128).T)
    v32["W_t2a"] = w["W_t2"][:128]; v32["W_t2b"] = w["W_t2"][128:]
    v32["b_t2"] = np.ascontiguousarray(w["b_t2"].reshape(2, 128).T)
    v32["W_sta"] = w["W_st"][:128]; v32["W_stb"] = w["W_st"][128:]
    v32["bstS1"] = (w["b_st"][:64] + 1.0).reshape(-1, 1)
    v32["bstSh"] = w["b_st"][64:].reshape(-1, 1)
    v32["W_tca"] = w["W_tc"][:128]; v32["W_tcb"] = w["W_tc"][128:]
    v32["b_tc"] = w["b_tc"].reshape(-1, 1)
    for k in ["b_fc", "b_c1", "b_c2", "b_r1", "b_r2", "b_cls", "b_reg"]:
        v32[k] = w[k].reshape(-1, 1)
    v32["W_r1f"] = w["W_r1"]; v32["W_r2f"] = w["W_r2"]; v32["W_regf"] = w["W_reg"]
    for li, (H, W, SR, WP) in enumerate(LEVELS):
        v32[f"qrep{li}"] = np.broadcast_to(Q_S[_perm(SR)][None, :],
                                           (128, NS)).copy()
        nio = np.arange(128, dtype=np.float32) % WP
        nio = np.where(nio < W, -nio, 5.0)
        v32[f"negiota{li}"] = nio.reshape(128, 1)
    v32["qfrep"] = np.broadcast_to(QF_R[None, :], (128, NR)).copy()
    v32["halfpi"] = np.full((128, 1), math.pi / 2.0, np.float32)
    v32["ident"] = np.eye(128, dtype=np.float32)
    cp32 = np.zeros((128, COLS32), np.float32)
    for name, (rows, off, cols) in LAY32.items():
        cp32[0:rows, off:off + cols] = v32[name]
    m["cpack32"] = cp32

    v16 = {}
    for k in ["W_q", "W_k", "W_v", "W_c1", "W_c2", "W_r1", "W_r2",
              "W_cls", "W_reg"]:
        v16[k] = w[k].astype(np.float16)
    v16["W_o_bf"] = w["W_o"].astype(np.float16)
    v16["ones_bf"] = np.ones((128, 1), np.float16)
    for li, (H, W, SR, WP) in enumerate(LEVELS):
        ob = np.zeros((2 * SR, 128), np.float16)
        for j in range(SR):
            ob[j, j * WP:j * WP + W] = 1.0          # hi block rows
            ob[SR + j, j * WP:j * WP + W] = 1.0     # lo block rows
        v16[f"onesbc{li}"] = ob
    cp16 = np.zeros((128, COLS16), np.float16)
    for name, (rows, off, cols) in LAY16.items():
        cp16[0:rows, off:off + cols] = v16[name]
    m["cpack16"] = cp16

    m["anchA"], m["anchB"] = _anch_pack(inp_slice)
    shared["cp32base"] = cp32
    shared["cpack16"] = cp16
    return {k: np.ascontiguousarray(np.asarray(v)) for k, v in m.items()}


def _anch_pack(inp_slice):
    a = np.asarray(inp_slice["inputs"], np.float32)  # (NB, NP, 3)
    aA = np.empty((128, NB * 3), np.float32)
    aB = np.empty((64, NB * 3), np.float32)
    for b in range(NB):
        aA[:, b * 3:(b + 1) * 3] = a[b, 0:128, :]
        aB[:, b * 3:(b + 1) * 3] = a[b, 128:192, :]
    return aA, aB


def make_in_maps(inputs):
    inputs = {k: np.asarray(v) for k, v in inputs.items()}
    nwfc2 = [_neg_wywfc2(np.asarray(inputs["W_fc"], np.float32), H)
             for H, W, SR, WP in LEVELS]
    shared = {}
    in_maps = []
    for c in range(N_CORES):
        sl = slice(c * NB, (c + 1) * NB)
        inp_slice = {k: (v[sl] if k in ("feat0", "feat1", "feat2", "inputs", "t")
                         else v) for k, v in inputs.items()}
        in_maps.append(_host_inputs(inp_slice, nwfc2, shared))
    return in_maps


def kernel(**inputs):
    if "prog" not in _CACHE:
        _CACHE["prog"] = _build_program()
    nc = _CACHE["prog"]
    in_maps = make_in_maps(inputs)
    res = bass_utils.run_bass_kernel_spmd(nc, in_maps,
                                          core_ids=list(range(N_CORES)))
    out = np.concatenate([res.results[c]["out"] for c in range(N_CORES)], axis=0)
    return np.ascontiguousarray(out.astype(np.float32))


# revision 51
# speedup vs baseline: 1.0009x; 1.0009x over previous
"""Trainium2 Bass kernel for nn_CLRerHead (CLRNet-style lane-detection head).

Sharding: data-parallel over batch. 32 batch items -> 8 cores x 4 items each.
Each core runs the full 3-level refinement for its 4 items.

Gather strategy ("negative tent" matmul gather): per level,
  grid_sample + roi-flatten + FC fuse into two PE matmul stages:
    RCPROJ_s (W, 64) = stackfeat_s(128, W).T @ nwfc2_s(128, 64)
      (y0/y1 corner rows stacked in K, -wy weights folded into nwfc2)
    f (64, 192)     += sum_G matmul(lhsT=RCPROJ_G(SR*W, 64), rhs=v_G)
  with v = min(|x - w| - 1, 0) the negative tent (signs cancel), built by
    PE broadcast matmul (block-diag ones lhsT over [hi;lo] fp16 rows of
    clamped x) -> ACT Abs(psum + (-iota mod W) bias) -> DVE min -> fp16.
  SR in {4,2,1} s-values stack into the 128 partitions per level, cutting
  tent elem-work and matmul rows by SR.
All feature/attention/head matmuls run fp16 operands with fp32 PSUM.
"""

import math
import numpy as np
import ml_dtypes
from contextlib import ExitStack

import concourse.bass as bass
import concourse.bacc as bacc
import concourse.mybir as mybir
import concourse.tile as tile
from concourse import bass_utils

dt = mybir.dt
AF = mybir.ActivationFunctionType
ALU = mybir.AluOpType

# ---------------- static problem config ----------------
IMG_W, IMG_H = 800.0, 320.0
NR, NS, NP, FC = 72, 36, 192, 64
N_STRIPS = NR - 1
ALPHA = IMG_H / IMG_W
SAMPLE_IDX = (np.linspace(0.0, 1.0, NS) * N_STRIPS).astype(np.int64)
PRIOR_FEAT_YS = np.flip(SAMPLE_IDX.astype(np.float32) / N_STRIPS).copy()
PRIOR_YS = np.linspace(1.0, 0.0, NR, dtype=np.float32)

N_CORES = 8
NB = 4
# (H, W, SR, WP): feat2, feat1, feat0; SR s-values stacked at stride WP
LEVELS = [(10, 25, 4, 32), (20, 50, 2, 64), (40, 100, 1, 100)]
PCH = [(0, 128), (128, 64)]
FP16 = dt.float16
F32 = dt.float32
NPTS = NP * NS          # 6912 tent points per (item, level)
CH = 512                # tent chunk free-width (one psum bank)
SG = 8                  # s-values per rc psum tile

Q_S = (1.0 - PRIOR_YS[SAMPLE_IDX[::-1]]).astype(np.float32)
QF_R = (1.0 - PRIOR_YS).astype(np.float32)


def _perm(SR):
    return [s for j in range(SR) for s in range(j, NS, SR)]


def _level_ytab(H):
    ys = PRIOR_FEAT_YS * (H - 1)
    y0 = np.clip(np.floor(ys).astype(np.int64), 0, H - 1)
    y1 = np.minimum(y0 + 1, H - 1)
    wy1 = (ys - y0).astype(np.float32)
    wy1 = np.where(y1 == y0, 0.0, wy1).astype(np.float32)
    wy0 = (1.0 - wy1).astype(np.float32)
    return y0, y1, wy0, wy1


def _neg_wywfc2(W_fc, H):
    """(128, NS*64): rows 0:64 = -wy0(s)*Wfc_s, rows 64:128 = -wy1(s)*Wfc_s."""
    _, _, wy0, wy1 = _level_ytab(H)
    out = np.zeros((128, NS * 64), np.float32)
    for s in range(NS):
        out[0:64, s * 64:(s + 1) * 64] = -wy0[s] * W_fc[s::NS, :]
        out[64:128, s * 64:(s + 1) * 64] = -wy1[s] * W_fc[s::NS, :]
    return out.astype(np.float16)


def _stackfeat_all(f, H, W, WP):
    """(128, NB*NS*WP): per item blocks of [feat[:, y0(s), :]; feat[:, y1(s)]],
    zero-padded to WP columns so PE pad rows compute to zero."""
    y0, y1, _, _ = _level_ytab(H)
    f16 = f.astype(np.float16)                      # (NB, 64, H, W)
    out = np.zeros((128, NB, NS, WP), np.float16)
    out[0:64, :, :, 0:W] = f16[:, :, y0, :].transpose(1, 0, 2, 3)
    out[64:128, :, :, 0:W] = f16[:, :, y1, :].transpose(1, 0, 2, 3)
    return out.reshape(128, NB * NS * WP)


# ---------------- packed const layouts (static) ----------------
def _pack_layout(entries):
    off = 0
    lay = {}
    for name, rows, cols in entries:
        lay[name] = (rows, off, cols)
        off += cols
    return lay, off


PACK32 = [
    ("sinargsT", 64, NB), ("W_t1", 64, 256), ("b_t1", 128, 2),
    ("W_t2a", 128, 256), ("W_t2b", 128, 256), ("b_t2", 128, 2),
    ("W_sta", 128, 128), ("W_stb", 128, 128), ("bstS1", 64, 1),
    ("bstSh", 64, 1), ("W_tca", 128, 64), ("W_tcb", 128, 64), ("b_tc", 64, 1),
    ("b_fc", 64, 1), ("b_c1", 64, 1), ("b_c2", 64, 1), ("b_r1", 64, 1),
    ("b_r2", 64, 1), ("b_cls", 2, 1), ("b_reg", 76, 1),
    ("W_r1f", 64, 64), ("W_r2f", 64, 64), ("W_regf", 64, 76),
    ("qrep0", 128, NS), ("qrep1", 128, NS), ("qrep2", 128, NS),
    ("qfrep", 128, NR), ("negiota0", 128, 1), ("negiota1", 128, 1),
    ("negiota2", 128, 1), ("halfpi", 128, 1), ("ident", 128, 128),
]
LAY32, COLS32 = _pack_layout(PACK32)

PACK16 = [
    ("W_q", 64, 64), ("W_k", 64, 64), ("W_v", 64, 64), ("W_o_bf", 64, 64),
    ("W_c1", 64, 64), ("W_c2", 64, 64), ("W_r1", 64, 64), ("W_r2", 64, 64),
    ("W_cls", 64, 2), ("W_reg", 64, 76), ("ones_bf", 128, 1),
    ("onesbc0", 8, 128), ("onesbc1", 4, 128), ("onesbc2", 2, 128),
]
LAY16, COLS16 = _pack_layout(PACK16)

_CACHE = {}


def _build_program(num_devices=N_CORES):
    nc = bacc.Bacc("TRN2", target_bir_lowering=False, debug=False,
                   num_devices=num_devices)
    D = {}

    def din(name, shape, dtype=F32):
        D[name] = nc.dram_tensor(name, list(shape), dtype, kind="ExternalInput")

    for li, (H, W, SR, WP) in enumerate(LEVELS):
        din(f"sfeat{li}", (128, NB * NS * WP), FP16)
        din(f"nwfc{li}", (128, NS * 64), FP16)
    din("cpack32", (128, COLS32))
    din("cpack16", (128, COLS16), FP16)
    din("anchA", (128, NB * 3))
    din("anchB", (64, NB * 3))

    out_t = nc.dram_tensor("out", [NB, NP, 78], F32, kind="ExternalOutput")

    with tile.TileContext(nc) as tc, ExitStack() as ex:
        cpool = ex.enter_context(tc.tile_pool(name="consts", bufs=1))
        state = ex.enter_context(tc.tile_pool(name="state", bufs=1))
        wk = ex.enter_context(tc.tile_pool(name="work", bufs=2))
        big = ex.enter_context(tc.tile_pool(name="big", bufs=2))
        ps = ex.enter_context(tc.tile_pool(name="ps", bufs=2, space="PSUM"))
        psf = ex.enter_context(tc.tile_pool(name="psf", bufs=2, space="PSUM"))
        psrc = ex.enter_context(tc.tile_pool(name="psrc", bufs=2, space="PSUM"))
        psbc = ex.enter_context(tc.tile_pool(name="psbc", bufs=2, space="PSUM"))

        C = {}
        SF, NW = {}, {}
        for li, (H, W, SR, WP) in enumerate(LEVELS):
            SF[li] = cpool.tile([128, NB * NS * WP], FP16, tag=f"sfeat{li}",
                                name=f"c_sfeat{li}")
            NW[li] = cpool.tile([128, NS * 64], FP16, tag=f"nwfc{li}",
                                name=f"c_nwfc{li}")
        cp32 = cpool.tile([128, COLS32], F32, tag="cpack32", name="cp32")
        cp16 = cpool.tile([128, COLS16], FP16, tag="cpack16", name="cp16")
        # DMA order: level-0-critical tensors first so compute starts early.
        nc.sync.dma_start(cp32[:], D["cpack32"].ap())
        nc.sync.dma_start(cp16[:], D["cpack16"].ap())
        nc.scalar.dma_start(NW[0][:], D["nwfc0"].ap())
        nc.scalar.dma_start(SF[0][:], D["sfeat0"].ap())
        for li in (1, 2):
            nc.scalar.dma_start(SF[li][:], D[f"sfeat{li}"].ap())
            nc.scalar.dma_start(NW[li][:], D[f"nwfc{li}"].ap())
        for name, (rows, off, cols) in LAY32.items():
            C[name] = cp32[0:rows, off:off + cols]
        for name, (rows, off, cols) in LAY16.items():
            C[name] = cp16[0:rows, off:off + cols]

        anchT = {}
        anchT[0] = state.tile([128, NB * 3], F32, tag="anchA", name="anchA_t")
        anchT[1] = state.tile([64, NB * 3], F32, tag="anchB", name="anchB_t")
        nc.sync.dma_start(anchT[0][:], D["anchA"].ap())
        nc.sync.dma_start(anchT[1][:], D["anchB"].ap())
        anch = {(b, ci): anchT[ci][:, b * 3:(b + 1) * 3]
                for b in range(NB) for ci in range(2)}



        # ---------------- time MLP (emitted in 4 parts, interleaved
        # with level-0 passT so its serial chain hides behind tent work) ----
        MS = {}

        def mlp_part1():
            sinT = wk.tile([64, NB], F32, tag="tm_sin", name="sinT")
            nc.scalar.activation(sinT[:], C["sinargsT"][:], AF.Sin)
            emb = []
            for m in range(2):
                p = ps.tile([128, NB], F32, tag="mm", name=f"p_emb{m}")
                nc.tensor.matmul(p[:], C["W_t1"][:, m * 128:(m + 1) * 128],
                                 sinT[:])
                x = state.tile([128, NB], F32, tag=f"emb{m}", name=f"emb{m}")
                nc.scalar.activation(x[:], p[:], AF.Identity,
                                     bias=C["b_t1"][:, m:m + 1])
                emb.append(x)
            MS["emb"] = emb

        def mlp_part2():
            emb = MS["emb"]
            for m in range(2):
                x = emb[m]
                sq = wk.tile([128, NB], F32, tag="tm_sq", name=f"sq{m}")
                nc.scalar.activation(sq[:], x[:], AF.Square)
                cu = wk.tile([128, NB], F32, tag="tm_cu", name=f"cu{m}")
                nc.vector.tensor_tensor(cu[:], sq[:], x[:], ALU.mult)
                nc.vector.tensor_scalar(cu[:], cu[:], 0.044715, None, ALU.mult)
                nc.vector.tensor_tensor(cu[:], cu[:], x[:], ALU.add)
                th = wk.tile([128, NB], F32, tag="tm_th", name=f"th{m}")
                nc.scalar.activation(th[:], cu[:], AF.Tanh,
                                     scale=float(np.sqrt(2.0 / np.pi)))
                nc.vector.tensor_scalar(th[:], th[:], 1.0, 0.5, ALU.add,
                                        ALU.mult)
                nc.vector.tensor_tensor(x[:], th[:], x[:], ALU.mult)

        def mlp_part3():
            emb = MS["emb"]
            tmb = []
            for m in range(2):
                p = ps.tile([128, NB], F32, tag="mm", name=f"p_tmb{m}")
                for k in range(2):
                    wt2 = C["W_t2a"] if k == 0 else C["W_t2b"]
                    nc.tensor.matmul(p[:], wt2[:, m * 128:(m + 1) * 128],
                                     emb[k][:], start=(k == 0), stop=(k == 1))
                x = state.tile([128, NB], F32, tag=f"tmb{m}", name=f"tmb{m}")
                nc.scalar.activation(x[:], p[:], AF.Identity,
                                     bias=C["b_t2"][:, m:m + 1])
                tmb.append(x)
            sil = []
            for m in range(2):
                # silu(x) = 0.5*x*(1 + tanh(x/2)) keeps the ACT stream inside
                # the silu/tanh/sin table set (no sigmoid-set load).
                sl = wk.tile([128, NB], F32, tag=f"tm_sil{m}", name=f"sil{m}")
                nc.scalar.activation(sl[:], tmb[m][:], AF.Tanh, scale=0.5)
                nc.vector.tensor_scalar(sl[:], sl[:], 1.0, 0.5, ALU.add,
                                        ALU.mult)
                nc.vector.tensor_tensor(sl[:], sl[:], tmb[m][:], ALU.mult)
                sil.append(sl)
            MS["tmb"], MS["sil"] = tmb, sil

        def mlp_part4():
            tmb, sil = MS["tmb"], MS["sil"]
            for j, (dst, bias) in enumerate([(scale1T, "bstS1"),
                                             (shiftT, "bstSh")]):
                p = ps.tile([64, NB], F32, tag="mm", name=f"p_ss{j}")
                for k in range(2):
                    wst = C["W_sta"] if k == 0 else C["W_stb"]
                    nc.tensor.matmul(p[:], wst[:, j * 64:(j + 1) * 64],
                                     sil[k][:], start=(k == 0), stop=(k == 1))
                nc.scalar.activation(dst[:], p[:], AF.Identity,
                                     bias=C[bias][:, 0:1])
            ptk = ps.tile([64, NB], F32, tag="mm", name="p_tok")
            for k in range(2):
                wtc = C["W_tca"] if k == 0 else C["W_tcb"]
                nc.tensor.matmul(ptk[:], wtc[:], tmb[k][:], start=(k == 0),
                                 stop=(k == 1))
            nc.scalar.activation(tokT[:], ptk[:], AF.Identity,
                                 bias=C["b_tc"][:, 0:1])

        scale1T = state.tile([64, NB], F32, tag="scale1T", name="scale1T")
        shiftT = state.tile([64, NB], F32, tag="shiftT", name="shiftT")
        tokT = state.tile([64, NB], F32, tag="tokT", name="tokT")

        # ---------------- helpers ----------------
        def gen_ab(b, scaleW, tagsfx, gate=None):
            res = []
            for ci, (p0, pn) in enumerate(PCH):
                A = anch[(b, ci)]
                th = A[:, 2:3]
                if gate is not None:
                    tg = wk.tile([pn, 1], F32, tag=f"tg{ci}{tagsfx}",
                                 name=f"tg{b}{ci}")
                    nc.vector.tensor_tensor(tg[:], A[:, 2:3], gate[0:pn, 0:1],
                                            ALU.add)
                    th = tg[:, 0:1]
                sn = wk.tile([pn, 1], F32, tag=f"sn{ci}{tagsfx}", name=f"sn{b}{ci}")
                cs = wk.tile([pn, 1], F32, tag=f"cs{ci}{tagsfx}", name=f"cs{b}{ci}")
                nc.scalar.activation(sn[:], th, AF.Sin, scale=math.pi)
                nc.scalar.activation(cs[:], th, AF.Sin, scale=-math.pi,
                                     bias=C["halfpi"][0:pn, 0:1])
                g = wk.tile([pn, 1], F32, tag=f"g{ci}{tagsfx}", name=f"g{b}{ci}")
                nc.vector.reciprocal(g[:], sn[:])
                nc.vector.tensor_tensor(g[:], cs[:], g[:], ALU.mult)
                nc.vector.tensor_scalar(g[:], g[:], 1000.0, -1000.0,
                                        ALU.min, ALU.max)
                nc.vector.tensor_scalar(g[:], g[:], ALPHA, None, ALU.mult)
                base = wk.tile([pn, 1], F32, tag=f"bs{ci}{tagsfx}", name=f"bs{b}{ci}")
                nc.vector.tensor_tensor(base[:], A[:, 0:1], g[:], ALU.mult)
                nc.vector.tensor_tensor(base[:], A[:, 1:2], base[:], ALU.subtract)
                aC = wk.tile([pn, 1], F32, tag=f"aC{ci}{tagsfx}", name=f"aC{b}{ci}")
                bC = wk.tile([pn, 1], F32, tag=f"bC{ci}{tagsfx}", name=f"bC{b}{ci}")
                nc.vector.tensor_scalar(aC[:], base[:], scaleW, None, ALU.mult)
                nc.vector.tensor_scalar(bC[:], g[:], scaleW, None, ALU.mult)
                res.append((aC, bC, base, g))
            return res

        # ---------------- main loop ----------------
        def emit_prefix(li, b, gate=None):
            """gen_ab + xf + transpose + clamp/split + rhsS collapse DMAs."""
            H, W, SR, WP = LEVELS[li]
            qrep = C[f"qrep{li}"]
            xfs = {}
            ab = gen_ab(b, float(W - 1), "a", gate=gate)
            for ci, (p0, pn) in enumerate(PCH):
                aC, bC, _, _ = ab[ci]
                xf = state.tile([pn, NS], F32, tag=f"xf{b}_{ci}",
                                name=f"xf{b}_{ci}_{li}")
                nc.vector.tensor_scalar(xf[:], qrep[0:pn, :],
                                        bC[:, 0:1], aC[:, 0:1],
                                        ALU.mult, ALU.add)
                xfs[ci] = xf
            ptx = ps.tile([NS, NP], F32, tag="mm", name=f"ptx{b}")
            for ci, (p0, pn) in enumerate(PCH):
                nc.tensor.transpose(ptx[:, p0:p0 + pn], xfs[ci][:],
                                    C["ident"][0:pn, 0:pn])
            xfc = wk.tile([NS, NP], F32, tag="xfc", bufs=4, name=f"xfc{b}")
            nc.vector.tensor_scalar(xfc[:], ptx[:], -2.0, float(W + 2),
                                    ALU.max, ALU.min)
            hi16 = wk.tile([NS, NP], FP16, tag="hi16", bufs=4, name=f"hi16{b}")
            nc.gpsimd.tensor_copy(hi16[:], xfc[:])
            lo16 = wk.tile([NS, NP], FP16, tag="lo16", bufs=4, name=f"lo16{b}")
            nc.gpsimd.tensor_tensor(lo16[:], xfc[:], hi16[:], ALU.subtract)
            rhsS = big.tile([2 * SR, NPTS // SR], FP16, tag="rhsS", bufs=4,
                            name=f"rhsS{b}")
            nc.sync.dma_start(rhsS[0:SR, :], hi16[:])
            nc.sync.dma_start(rhsS[SR:2 * SR, :], lo16[:])
            return rhsS

        pre = {(0, b): emit_prefix(0, b) for b in range(NB)}

        for li, (H, W, SR, WP) in enumerate(LEVELS):
            is_last = li == len(LEVELS) - 1
            sfeat, nwfc = SF[li], NW[li]
            NQL = NPTS // SR      # tent columns (1728/3456/6912)
            NGRP = NS // SR       # fps groups (9/18/36)
            nch = (NQL + CH - 1) // CH
            nio = C[f"negiota{li}"]
            obc = C[f"onesbc{li}"]

            # pass T: tent + PROJ + fps + fT per item
            fTs = {}
            gates = {}
            def passT_item(b):
                rhsS = pre.pop((li, b))
                # PROJ: one corner-stacked matmul per s into rc psum tiles
                rcs = []
                nloc = SG // SR   # fps groups per rc tile (2/4/8)
                for k in range((NS + SG - 1) // SG):
                    smax = min((k + 1) * SG, NS)
                    qlmax = (smax - 1) // SR - k * nloc + 1
                    rc = psrc.tile([128, SG * 64], F32, tag="rc_ps",
                                   name=f"rc{b}_{k}")
                    for s in range(k * SG, smax):
                        j = s % SR
                        ql = s // SR - k * nloc
                        nc.tensor.matmul(
                            rc[j * WP:j * WP + WP, ql * 64:(ql + 1) * 64],
                            sfeat[:, (b * NS + s) * WP:(b * NS + s + 1) * WP],
                            nwfc[:, s * 64:(s + 1) * 64],
                            tile_position=(0, j * WP))
                    rcsb = wk.tile([128, SG * 64 // SR], FP16, tag="rcsb",
                                   bufs=6, name=f"rcsb{b}_{k}")
                    nc.vector.tensor_copy(rcsb[0:SR * WP, 0:qlmax * 64],
                                          rc[0:SR * WP, 0:qlmax * 64])
                    rcs.append(rcsb)

                # tent chunks: PE bcast -> ACT abs -> DVE min, 512-wide
                # (full psum bank); min lands in one contiguous v16 tile so
                # fps can slice at any 192 offset.
                v16 = big.tile([128, NPTS], FP16, tag="v16", bufs=2,
                               name=f"v16{b}")
                for c in range(nch):
                    cw = min(CH, NQL - c * CH)
                    bc = psbc.tile([128, CH], F32, tag="bc", name=f"bc{b}_{c}")
                    nc.tensor.matmul(bc[0:SR * WP, 0:cw],
                                     obc[0:2 * SR, 0:SR * WP],
                                     rhsS[0:2 * SR, c * CH:c * CH + cw])
                    d16 = wk.tile([128, CH], FP16, tag="d16", bufs=4,
                                  name=f"d16{b}_{c}")
                    nc.scalar.activation(d16[0:SR * WP, 0:cw], bc[0:SR * WP, 0:cw],
                                         AF.Abs, bias=nio[0:SR * WP, 0:1])
                    nc.vector.tensor_scalar(v16[0:SR * WP, c * CH:c * CH + cw],
                                            d16[0:SR * WP, 0:cw], 1.0, 0.0,
                                            ALU.subtract, ALU.min)

                # fps accumulation over groups
                fps = psf.tile([64, NP], F32, tag="f_ps", name=f"fps{b}")
                for G in range(NGRP):
                    k = G // nloc
                    ql = G % nloc
                    nc.tensor.matmul(
                        fps[:],
                        rcs[k][0:SR * WP, ql * 64:(ql + 1) * 64],
                        v16[0:SR * WP, G * NP:(G + 1) * NP],
                        start=(G == 0), stop=(G == NGRP - 1))

                fT = wk.tile([64, NP], F32, tag="fT", bufs=7, name=f"fT{b}")
                nc.scalar.activation(fT[:], fps[:], AF.Relu, bias=C["b_fc"][:, 0:1])
                fTs[b] = fT

            # pass A: attention per item (psum packed into 4 tiles)
            def passA_item(b):
                fT = fTs[b]
                nc.vector.tensor_scalar(fT[:], fT[:], tokT[:, b:b + 1], None,
                                        ALU.add)
                fT16 = wk.tile([64, NP], FP16, tag="fT16", bufs=7,
                               name=f"fT16{b}")
                nc.gpsimd.tensor_copy(fT16[:], fT[:])
                fTs[b] = (fT, fT16)
                qk = ps.tile([128, NP], F32, tag="mm", name=f"qk{b}")
                nc.tensor.matmul(qk[0:64, :], C["W_q"][:], fT16[:])
                nc.tensor.matmul(qk[64:128, :], C["W_k"][:], fT16[:],
                                 tile_position=(0, 64))
                qT = wk.tile([64, NP], FP16, tag="qT", name=f"qT{b}")
                nc.vector.tensor_scalar(qT[:], qk[0:64, :], 0.125, None,
                                        ALU.mult)
                kT = wk.tile([64, NP], FP16, tag="kT", name=f"kT{b}")
                nc.vector.tensor_copy(kT[:], qk[64:128, :])
                vp = ps.tile([128, 128], F32, tag="mm", name=f"vp{b}")
                nc.tensor.matmul(vp[0:128, 0:64], fT16[:, 0:128], C["W_v"][:])
                nc.tensor.matmul(vp[0:64, 64:128], fT16[:, 128:192], C["W_v"][:])
                vn = []
                for ci, (p0, pn) in enumerate(PCH):
                    vt = wk.tile([pn, 64], FP16, tag=f"vn{ci}", name=f"vn{b}{ci}")
                    nc.vector.tensor_copy(vt[:], vp[0:pn, ci * 64:ci * 64 + 64])
                    vn.append(vt)
                sp = ps.tile([128, 2 * NP], F32, tag="mm", name=f"sp{b}")
                nc.tensor.matmul(sp[0:128, 0:NP], kT[:, 0:128], qT[:])
                nc.tensor.matmul(sp[0:64, NP:2 * NP], kT[:, 128:192], qT[:])
                est = []
                for ci, (p0, pn) in enumerate(PCH):
                    e = wk.tile([pn, NP], FP16, tag=f"est{ci}", name=f"est{b}{ci}")
                    nc.scalar.activation(e[:], sp[0:pn, ci * NP:ci * NP + NP],
                                         AF.Exp)
                    est.append(e)
                zav = ps.tile([128, 2 * NP], F32, tag="mm", name=f"zav{b}")
                for ci, (p0, pn) in enumerate(PCH):
                    nc.tensor.matmul(zav[0:1, 0:NP], C["ones_bf"][0:pn, 0:1],
                                     est[ci][:], start=(ci == 0), stop=(ci == 1))
                rrow = wk.tile([1, NP], F32, tag="rrow", name=f"rrow{b}")
                nc.vector.reciprocal(rrow[:], zav[0:1, 0:NP])
                rbc = wk.tile([64, NP], F32, tag="rbc", name=f"rbc{b}")
                nc.gpsimd.partition_broadcast(rbc[:], rrow[0:1, :], channels=64)
                for ci in range(2):
                    nc.tensor.matmul(zav[64:128, 0:NP], vn[ci][:], est[ci][:],
                                     start=(ci == 0), stop=(ci == 1),
                                     tile_position=(0, 64))
                avsb = wk.tile([64, NP], FP16, tag="avsb", name=f"avsb{b}")
                nc.vector.tensor_copy(avsb[:], zav[64:128, 0:NP])
                nc.tensor.matmul(zav[0:64, NP:2 * NP], C["W_o_bf"][:], avsb[:])
                t1 = wk.tile([64, NP], F32, tag="attnt", name=f"t1{b}")
                nc.vector.tensor_tensor(t1[:], zav[0:64, NP:2 * NP], rbc[:],
                                        ALU.mult)
                nc.vector.tensor_tensor(fT[:], fT[:], t1[:], ALU.add)
                # FiLM
                nc.vector.tensor_scalar(fT[:], fT[:], scale1T[:, b:b + 1],
                                        shiftT[:, b:b + 1], ALU.mult, ALU.add)
                gates[b] = est[0]

            # pass H: heads per item (+ next level prefix / final outputs)
            def passH_item(b):
                fT, _ = fTs[b]

                def head_mm(wname, bias, src, relu=True, out_p=64, out_dt=FP16):
                    p = ps.tile([128, NP], F32, tag="mm", name=f"p_{wname}{b}")
                    nc.tensor.matmul(p[0:out_p, :], C[wname][:], src[:])
                    o = wk.tile([out_p, NP], out_dt, tag=f"hd_{wname}",
                                name=f"{wname}o{b}")
                    nc.scalar.activation(o[:], p[0:out_p, :],
                                         AF.Relu if relu else AF.Identity,
                                         bias=C[bias][:, 0:1])
                    return o

                pr = ps.tile([128, NP], F32, tag="mm", name=f"pr{b}")
                nc.tensor.matmul(pr[0:64, :], C["W_r1f"][:], fT[:])
                r1 = wk.tile([64, NP], F32, tag="hd_r1", name=f"r1o{b}")
                nc.vector.tensor_scalar(r1[:], pr[0:64, :], C["b_r1"][:, 0:1],
                                        0.0, ALU.add, ALU.max)
                nc.tensor.matmul(pr[64:128, :], C["W_r2f"][:], r1[:],
                                 tile_position=(0, 64))
                r2 = wk.tile([64, NP], F32, tag="hd_r2", name=f"r2o{b}")
                nc.vector.tensor_scalar(r2[:], pr[64:128, :], C["b_r2"][:, 0:1],
                                        0.0, ALU.add, ALU.max)
                # non-last levels only consume reg[..., 0:3] (anchor
                # deltas): shrink the reg head to 3 columns and update
                # anchors straight from the transpose psum (no rn copy).
                nreg = 76 if is_last else 3
                p_regT = ps.tile([128, NP], F32, tag="mm", name=f"p_regT{b}")
                nc.tensor.matmul(p_regT[0:nreg, :], C["W_regf"][:, 0:nreg],
                                 r2[:])
                regT = wk.tile([nreg, NP], F32, tag="hd_regf",
                               name=f"regTo{b}")
                nc.scalar.activation(regT[:], p_regT[0:nreg, :], AF.Identity,
                                     bias=C["b_reg"][0:nreg, 0:1])

                rns = {}
                pt = ps.tile([128, 2 * 76], F32, tag="mm", name=f"p_rt{b}")
                for ci, (p0, pn) in enumerate(PCH):
                    nc.tensor.transpose(pt[0:pn, ci * nreg:ci * nreg + nreg],
                                        regT[:, p0:p0 + pn],
                                        C["ident"][0:nreg, 0:nreg])
                    A = anch[(b, ci)]
                    if is_last:
                        rn = state.tile([pn, 76], F32, tag=f"regn{b}_{ci}",
                                        name=f"regn{b}_{ci}_{li}")
                        nc.vector.tensor_copy(rn[:],
                                              pt[0:pn, ci * 76:ci * 76 + 76])
                        nc.vector.tensor_tensor(A[:, :], A[:, :], rn[:, 0:3],
                                                ALU.add)
                        rns[ci] = rn
                    else:
                        nc.vector.tensor_tensor(
                            A[:, :], A[:, :],
                            pt[0:pn, ci * nreg:ci * nreg + 3], ALU.add)

                if is_last:
                    stg = {ci: state.tile([pn, 78], F32, tag=f"stg{b}{ci}",
                                          name=f"stg{b}{ci}")
                           for ci, (p0, pn) in enumerate(PCH)}
                    fT16f = wk.tile([64, NP], FP16, tag="fT16f",
                                    name=f"fT16f{b}")
                    nc.gpsimd.tensor_copy(fT16f[:], fT[:])
                    c1 = head_mm("W_c1", "b_c1", fT16f)
                    c2 = head_mm("W_c2", "b_c2", c1)
                    clsT = head_mm("W_cls", "b_cls", c2, relu=False, out_p=2,
                                   out_dt=F32)
                    ab = gen_ab(b, 1.0, "o")
                    ptc = ps.tile([128, 4], F32, tag="mm", name=f"p_ct{b}")
                    for ci, (p0, pn) in enumerate(PCH):
                        nc.tensor.transpose(ptc[0:pn, ci * 2:ci * 2 + 2],
                                            clsT[:, p0:p0 + pn],
                                            C["ident"][0:2, 0:2])
                        nc.vector.tensor_copy(stg[ci][:, 0:2],
                                              ptc[0:pn, ci * 2:ci * 2 + 2])
                        A = anch[(b, ci)]
                        rn = rns[ci]
                        nc.gpsimd.tensor_copy(stg[ci][:, 2:5], A[:, :])
                        nc.gpsimd.tensor_copy(stg[ci][:, 5:6], rn[:, 3:4])
                        _, _, base, g = ab[ci]
                        nc.vector.tensor_scalar(stg[ci][:, 6:78],
                                                C["qfrep"][0:pn, 0:NR],
                                                g[:, 0:1], base[:, 0:1],
                                                ALU.mult, ALU.add)
                        nc.vector.tensor_tensor(stg[ci][:, 6:78],
                                                stg[ci][:, 6:78], rn[:, 4:76],
                                                ALU.add)
                        nc.sync.dma_start(out_t.ap()[b, p0:p0 + pn, 0:78],
                                          stg[ci][:])

            # interleaved schedule: T0 T1 A0 T2 A1 T3 A2 H0 A3 H1 H2 H3
            if li == 0:
                mlp_part1()
                passT_item(0); mlp_part2()
                passT_item(1); mlp_part3()
                passT_item(2); mlp_part4()
                passT_item(3)
                passA_item(0)
                passA_item(1)
                passA_item(2)
            else:
                passT_item(0); passT_item(1)
                passA_item(0)
                passT_item(2)
                passA_item(1)
                passT_item(3)
                passA_item(2)
            passH_item(0)
            passA_item(3)
            passH_item(1)
            if not is_last:
                ez = wk.tile([128, 1], F32, tag="ezgate", name=f"ez{li}")
                nc.vector.tensor_scalar(ez[:], gates[3][0:128, 0:1], 0.0, None,
                                        ALU.mult)
                pre[(li + 1, 0)] = emit_prefix(li + 1, 0, gate=ez)
                pre[(li + 1, 1)] = emit_prefix(li + 1, 1, gate=ez)
            passH_item(2); passH_item(3)
            if not is_last:
                pre[(li + 1, 2)] = emit_prefix(li + 1, 2, gate=ez)
                pre[(li + 1, 3)] = emit_prefix(li + 1, 3, gate=ez)

    nc.compile()
    _CACHE.pop("regn", None)
    return nc


def _host_inputs(inp_slice, nwfc2, shared):
    m = dict(shared)
    for li, key in enumerate(["feat2", "feat1", "feat0"]):
        f = np.asarray(inp_slice[key], np.float32)
        H, W, SR, WP = LEVELS[li]
        m[f"sfeat{li}"] = _stackfeat_all(f, H, W, WP)
        m[f"nwfc{li}"] = nwfc2[li]
    w = {k: np.asarray(v, np.float32) for k, v in inp_slice.items()
         if k.startswith(("W_", "b_"))}

    half = FC // 2
    freqs = np.exp(np.arange(half, dtype=np.float32)
                   * (-math.log(10000.0) / (half - 1)))
    ang = np.asarray(inp_slice["t"]).astype(np.float32)[:, None] * freqs[None, :]
    full = np.concatenate([ang, ang + math.pi / 2.0], axis=1)
    full = np.mod(full + math.pi, 2.0 * math.pi) - math.pi

    if "cp32base" in shared:
        m.pop("cp32base", None)
        cp32 = shared["cp32base"].copy()
        rows, off, cols = LAY32["sinargsT"]
        cp32[0:rows, off:off + cols] = np.ascontiguousarray(full.T)
        m["cpack32"] = cp32
        m["anchA"], m["anchB"] = _anch_pack(inp_slice)
        return {k: np.ascontiguousarray(np.asarray(v)) for k, v in m.items()}

    v32 = {}
    v32["sinargsT"] = np.ascontiguousarray(full.T)
    v32["W_t1"] = w["W_t1"]
    v32["b_t1"] = np.ascontiguousarray(w["b_t1"].reshape(2, 128).T)
    v32["W_t2a"] = w["W_t2"][:128]; v32["W_t2b"] = w["W_t2"][128:]
    v32["b_t2"] = np.ascontiguousarray(w["b_t2"].reshape(2, 128).T)
    v32["W_sta"] = w["W_st"][:128]; v32["W_stb"] = w["W_st"][128:]
    v32["bstS1"] = (w["b_st"][:64] + 1.0).reshape(-1, 1)
    v32["bstSh"] = w["b_st"][64:].reshape(-1, 1)
    v32["W_tca"] = w["W_tc"][:128]; v32["W_tcb"] = w["W_tc"][128:]
    v32["b_tc"] = w["b_tc"].reshape(-1, 1)
    for k in ["b_fc", "b_c1", "b_c2", "b_r1", "b_r2", "b_cls", "b_reg"]:
        v32[k] = w[k].reshape(-1, 1)
    v32["W_r1f"] = w["W_r1"]; v32["W_r2f"] = w["W_r2"]; v32["W_regf"] = w["W_reg"]
    for li, (H, W, SR, WP) in enumerate(LEVELS):
        v32[f"qrep{li}"] = np.broadcast_to(Q_S[_perm(SR)][None, :],
                                           (128, NS)).copy()
        nio = np.arange(128, dtype=np.float32) % WP
        nio = np.where(nio < W, -nio, 5.0)
        v32[f"negiota{li}"] = nio.reshape(128, 1)
    v32["qfrep"] = np.broadcast_to(QF_R[None, :], (128, NR)).copy()
    v32["halfpi"] = np.full((128, 1), math.pi / 2.0, np.float32)
    v32["ident"] = np.eye(128, dtype=np.float32)
    cp32 = np.zeros((128, COLS32), np.float32)
    for name, (rows, off, cols) in LAY32.items():
        cp32[0:rows, off:off + cols] = v32[name]
    m["cpack32"] = cp32

    v16 = {}
    for k in ["W_q", "W_k", "W_v", "W_c1", "W_c2", "W_r1", "W_r2",
              "W_cls", "W_reg"]:
        v16[k] = w[k].astype(np.float16)
    v16["W_o_bf"] = w["W_o"].astype(np.float16)
    v16["ones_bf"] = np.ones((128, 1), np.float16)
    for li, (H, W, SR, WP) in enumerate(LEVELS):
        ob = np.zeros((2 * SR, 128), np.float16)
        for j in range(SR):
            ob[j, j * WP:j * WP + W] = 1.0          # hi block rows
            ob[SR + j, j * WP:j * WP + W] = 1.0     # lo block rows
        v16[f"onesbc{li}"] = ob
    cp16 = np.zeros((128, COLS16), np.float16)
    for name, (rows, off, cols) in LAY16.items():
        cp16[0:rows, off:off + cols] = v16[name]
    m["cpack16"] = cp16

    m["anchA"], m["anchB"] = _anch_pack(inp_slice)
    shared["cp32base"] = cp32
    shared["cpack16"] = cp16
    return {k: np.ascontiguousarray(np.asarray(v)) for k, v in m.items()}


def _anch_pack(inp_slice):
    a = np.asarray(inp_slice["inputs"], np.float32)  # (NB, NP, 3)
    aA = np.empty((128, NB * 3), np.float32)
    aB = np.empty((64, NB * 3), np.float32)
    for b in range(NB):
        aA[:, b * 3:(b + 1) * 3] = a[b, 0:128, :]
        aB[:, b * 3:(b + 1) * 3] = a[b, 128:192, :]
    return aA, aB


def make_in_maps(inputs):
    inputs = {k: np.asarray(v) for k, v in inputs.items()}
    nwfc2 = [_neg_wywfc2(np.asarray(inputs["W_fc"], np.float32), H)
             for H, W, SR, WP in LEVELS]
    shared = {}
    in_maps = []
    for c in range(N_CORES):
        sl = slice(c * NB, (c + 1) * NB)
        inp_slice = {k: (v[sl] if k in ("feat0", "feat1", "feat2", "inputs", "t")
                         else v) for k, v in inputs.items()}
        in_maps.append(_host_inputs(inp_slice, nwfc2, shared))
    return in_maps


def kernel(**inputs):
    if "prog" not in _CACHE:
        _CACHE["prog"] = _build_program()
    nc = _CACHE["prog"]
    in_maps = make_in_maps(inputs)
    res = bass_utils.run_bass_kernel_spmd(nc, in_maps,
                                          core_ids=list(range(N_CORES)))
    out = np.concatenate([res.results[c]["out"] for c in range(N_CORES)], axis=0)
    return np.ascontiguousarray(out.astype(np.float32))
